# revision 1
# baseline (speedup 1.0000x reference)
"""Trainium2 Bass kernel for MultiHeadSelfAttention (RoPE + causal softmax).

Problem (hardcoded):
  x: (2, 2048, 512) f32, w_qkv: (1536, 512) f32, w_o: (512, 512) f32
  D_MODEL=512, N_HEADS=16, HEAD_DIM=32, ROPE_BASE=10000, causal.

Sharding: tensor-parallel over heads. Core c owns heads (2c, 2c+1) for both
batches. Each core computes its heads' q/k/v projections from the full x,
attention, and a Megatron-style row-parallel partial of the output
projection (out @ w_o.T restricted to its 64 input features). The host sums
the 8 partials (the row-parallel "unshard").

On-core layout highlights:
  - x is passed pre-transposed (xT [512, 4096]) so the d_model contraction
    sits on the partition axis for both projection orientations.
  - q,k are produced transposed ([feat, row]) and RoPE'd in that layout
    (rotate_half done with a block-diag permutation matmul on the PE).
  - scores are computed transposed (S.T [keys, queries]) so softmax'd P
    feeds the av matmul as weights without any transpose.
  - causal mask applied by accumulating -240*max(0, k-q) into the scores
    PSUM via a rank-128 A.T@B matmul of constant triangular matrices.
  - exp on the scalar engine (PSUM -> bf16 SBUF), with 1/sqrt(hd) folded
    into the activation scale. No max-subtraction (scores are provably
    small for this problem's scale).
  - row sums for softmax come from an extra all-ones column appended to v.
  - all big matmuls run float32r (1 cycle/row when N>=256) or bf16.
"""

import sys
import math
from contextlib import ExitStack

sys.path.insert(0, "/opt/trn_rl_repo")

import numpy as np
import ml_dtypes

import concourse.bass as bass
import concourse.tile as tile
from concourse import bacc, mybir
from concourse.bass_utils import run_bass_kernel_spmd

F32 = mybir.dt.float32
F32R = mybir.dt.float32r
BF16 = mybir.dt.bfloat16
EXP = mybir.ActivationFunctionType.Exp

B = 2
T = 2048
D = 512
NH = 16
HD = 32
NCORES = 8
R = B * T            # 4096 rows, row = b*T + t
NHL = NH // NCORES   # 2 heads per core
KC = T // 128        # 16 key chunks per batch
SCALE = 1.0 / math.sqrt(HD)
MASK_VAL = -240.0

def _bcast_free(ap_2d, n_inner):
    """[P, n] -> [P, n, n_inner] AP with the inner dim broadcast (step 0)."""
    return bass.AP(
        tensor=ap_2d.tensor,
        offset=ap_2d.offset,
        ap=list(ap_2d.ap[:-1]) + [list(ap_2d.ap[-1]), [0, n_inner]],
    )


def _emit(tc, io, loop_k=1):
    nc = tc.nc
    with ExitStack() as ctx:
        cpool = ctx.enter_context(tc.tile_pool(name="consts", bufs=1))
        mpool = ctx.enter_context(tc.tile_pool(name="main", bufs=1))
        spool = ctx.enter_context(tc.tile_pool(name="small", bufs=3))
        ppool = ctx.enter_context(tc.tile_pool(name="pk", bufs=2))
        # single PSUM pool, tags shared across phases (8 banks total):
        #   tagA [128,1024] x2 = 4 banks  (qk-proj / scores)
        #   tagB [128,512]  x2 = 2 banks  (shift/vT/vtr/atps/outps)
        #   tagC [128,8,33] x2 = 2 banks  (av accumulator groups)
        psum = ctx.enter_context(tc.tile_pool(name="psum", bufs=1, space="PSUM"))

        def tile_a():
            return psum.tile([128, 1024], F32, tag="A", bufs=2, name="psA")

        def tile_b(p=128, w=512):
            return psum.tile([p, w], F32, tag="B", bufs=2, name="psB")

        def tile_c():
            return psum.tile([128, 8, HD + 1], F32, tag="C", bufs=2, name="psC")

        # ---- constants (batched DMAs, spread over issue queues) ----
        wo = cpool.tile([64, 512], BF16, tag="wo")
        nc.scalar.dma_start(out=wo, in_=io["woT"])
        cmix = cpool.tile([128, 384], F32R, tag="cmix")
        nc.gpsimd.dma_start(out=cmix, in_=io["consts1"])
        permt = cmix[:, 0:128]
        trilA = cmix[:, 128:256]
        trilB = cmix[:, 256:384]
        ident = cpool.tile([128, 128], F32, tag="ident")
        nc.gpsimd.dma_start(out=ident, in_=io["ident"])
        wqkv = []
        for dc in range(4):
            w_t = cpool.tile([128, 192], F32R, tag=f"wqkv{dc}")
            nc.gpsimd.dma_start(out=w_t, in_=io["wqkvT"][dc * 128:(dc + 1) * 128, :])
            wqkv.append(w_t)

        # ---- persistent activations ----
        qkr = mpool.tile([128, R], F32R, tag="qkr")          # RoPE'd qT/kT
        ka = mpool.tile([64, R], F32R, tag="ka")             # k-half, base-aligned
        vall = mpool.tile([128, R // 128, NHL, HD + 1], BF16, tag="vall")
        ao = mpool.tile([128, B, KC, NHL, HD], BF16, tag="ao")  # attnout natural
        aoT = mpool.tile([64, R], BF16, tag="aoT")          # attnout transposed
        cosw = mpool.tile([128, T], F32, tag="cosw")        # one batch (shared)
        sinw = mpool.tile([128, T], F32, tag="sinw")

        nc.vector.memset(vall[:, :, :, HD:HD + 1], 1.0)     # softmax-sum column
        identb = cpool.tile([128, 128], BF16, tag="identb")
        nc.vector.tensor_copy(identb, ident)                # bf16 identity

        def emit_proj(bb):
            xt = [mpool.tile([128, T], F32R, tag=f"xt{dc}", bufs=1,
                             name=f"xt{dc}") for dc in range(4)]
            for j in range(4):
                for dc in range(4):
                    nc.sync.dma_start(
                        out=xt[dc][:, j * 512:(j + 1) * 512],
                        in_=io["xT"][dc * 128:(dc + 1) * 128,
                                     bb * T + j * 512:bb * T + (j + 1) * 512],
                    )
            if bb == 0:
                # after xt(b0) so x never queues behind these on the DMA rings
                nc.scalar.dma_start(out=cosw, in_=io["cosw"])
                nc.scalar.dma_start(out=sinw, in_=io["sinw"])

            for jl in range(4):
                colb = slice(jl * 512, (jl + 1) * 512)          # batch-local
                cols = slice(bb * T + jl * 512, bb * T + (jl + 1) * 512)
                # qT/kT projection: [feat, row] = wqkT.T @ xT
                qk_ps = tile_b()
                for dc in range(4):
                    nc.tensor.matmul(
                        qk_ps, wqkv[dc][:, 0:128], xt[dc][:, colb],
                        start=(dc == 0), stop=(dc == 3),
                    )
                # rotate_half via block-diag permutation (needs SBUF copy);
                # batch 0's copies ride the still-idle scalar engine
                qks = spool.tile([128, 512], F32R, tag="qks")
                if bb == 0:
                    nc.scalar.copy(qks, qk_ps)
                else:
                    nc.vector.tensor_copy(qks, qk_ps)
                sh_ps = tile_b()
                nc.tensor.matmul(sh_ps, permt, qks, start=True,
                                 stop=True)
                # qkr = qk*cos + shifted*sin_signed
                t1 = spool.tile([128, 512], F32, tag="t1")
                nc.vector.tensor_mul(t1, sh_ps, sinw[:, colb])
                nc.vector.tensor_mul(qkr[:, cols], qk_ps, cosw[:, colb])
                nc.vector.tensor_add(qkr[:, cols], qkr[:, cols], t1)
                # partition-aligned copy of the k rows (matmul requires lhsT
                # and rhs to share a base partition)
                nc.gpsimd.tensor_copy(ka[:, cols], qkr[64:128, cols])

                # vT projection: [feat, row]
                vt_ps = tile_b(64)
                for dc in range(4):
                    nc.tensor.matmul(
                        vt_ps, wqkv[dc][:, 128:192], xt[dc][:, colb],
                        start=(dc == 0), stop=(dc == 3),
                    )
                vt_sb = spool.tile([64, 512], BF16, tag="vtsb")
                if bb == 0:
                    nc.scalar.copy(vt_sb, vt_ps)
                else:
                    nc.vector.tensor_copy(vt_sb, vt_ps)
                # transpose v back to natural [row, feat] (bf16 on copy-out);
                # 4 transposes share one PSUM bank (disjoint 64-col regions)
                vtr_ps = psum.tile([128, 256], BF16, tag="B", bufs=2,
                                   name="psBv")
                for jj in range(4):
                    nc.tensor.transpose(
                        vtr_ps[:, jj * 64:(jj + 1) * 64],
                        vt_sb[:, jj * 128:(jj + 1) * 128],
                        identb[0:64, 0:64],
                    )
                for jj in range(4):
                    nc.vector.tensor_copy(
                        vall[:, bb * KC + jl * 4 + jj, :, 0:HD],
                        vtr_ps[:, jj * 64:(jj + 1) * 64])

        def emit_attention(bb, hh):
            if True:
                qrow = 32 * hh            # q rows in qkr
                krow = 32 * hh            # k rows in ka
                ppks = []
                pavs = {}

                def av_column(qc):
                    # av column for qc (P rows kc<=qc all exist);
                    # 8 query chunks per PSUM bank, normalized per group
                    g = qc // 8
                    if qc % 8 == 0:
                        pavs[g] = tile_c()
                    slot = pavs[g][:, qc % 8, :]
                    for kp in range(qc + 1):
                        nc.tensor.matmul(
                            slot,
                            ppks[kp][:, 128 * (qc - kp):128 * (qc - kp) + 128],
                            vall[:, bb * KC + kp, hh, :],
                            start=(kp == 0), stop=(kp == qc),
                        )
                    if qc % 8 == 7:
                        # normalize this group: attnout = av / l
                        pav = pavs[g]
                        rl = spool.tile([128, 8, 1], F32, tag="rl")
                        nc.vector.reciprocal(rl, pav[:, :, HD:HD + 1])
                        nc.vector.tensor_mul(
                            ao[:, bb, g * 8:(g + 1) * 8, hh, :],
                            pav[:, :, 0:HD],
                            _bcast_free(rl[:, :, 0], HD),
                        )

                # av columns trail the score/exp stream by 2 key chunks so
                # the PE never stalls waiting for the exp it just queued
                for kc in range(KC + 2):
                    if kc < KC:
                        n_kc = T - 128 * kc
                        # narrower first tile on the very first pair so the
                        # first exp fires one proj chunk earlier
                        cw = 1024
                        kslc = slice(bb * T + 128 * kc, bb * T + 128 * (kc + 1))
                        # per-kc P tile: precise deps (av reads never block
                        # later exps) and half the packed-tile footprint
                        ppk = ppool.tile([128, n_kc], BF16, tag=f"ppk{kc}",
                                         bufs=(2 if kc < 5 else 1),
                                         name=f"ppk{kc}")
                        ppks.append(ppk)
                        for c0 in range(0, n_kc, cw):
                            nt = min(cw, n_kc - c0)
                            sc_ps = tile_a()
                            for c in range(c0, c0 + nt, 512):
                                ln = min(512, n_kc - c)
                                qslc = slice(bb * T + 128 * kc + c,
                                             bb * T + 128 * kc + c + ln)
                                nc.tensor.matmul(
                                    sc_ps[:, c - c0:c - c0 + ln],
                                    ka[krow:krow + 32, kslc],
                                    qkr[qrow:qrow + 32, qslc],
                                    start=True, stop=(c > 0),
                                    skip_group_check=True,
                                )
                            if c0 == 0:
                                # causal mask on the diagonal 128x128 block:
                                # accumulates -240*max(0, k-q)
                                nc.tensor.matmul(
                                    sc_ps[:, 0:128], trilA, trilB,
                                    start=False, stop=True,
                                    skip_group_check=True,
                                )
                            nc.scalar.activation(
                                out=ppk[:, c0:c0 + nt],
                                in_=sc_ps[:, 0:nt],
                                func=EXP, scale=SCALE,
                            )
                    if kc >= 2:
                        av_column(kc - 2)

        def emit_epilogue(bb, last):
            # transpose attnout group g, then immediately out-proj its 4
            # row chunks so the tail drains incrementally
            for g in range(4):
                at_ps = psum.tile([64, 512], BF16, tag="B", bufs=2,
                                  name="psBt")
                for jj in range(4):
                    qc = g * 4 + jj
                    nc.tensor.transpose(
                        at_ps[:, jj * 128:(jj + 1) * 128],
                        ao[:, bb, qc, :, :].rearrange("p a b -> p (a b)"),
                        identb,
                    )
                if last and g % 2 == 1:
                    nc.scalar.copy(
                        aoT[:, bb * T + g * 512:bb * T + (g + 1) * 512],
                        at_ps)
                else:
                    nc.vector.tensor_copy(
                        aoT[:, bb * T + g * 512:bb * T + (g + 1) * 512],
                        at_ps)
                for qc in range(g * 4, g * 4 + 4):
                    rc = bb * KC + qc
                    out_ps = tile_b()
                    nc.tensor.matmul(
                        out_ps, aoT[:, rc * 128:(rc + 1) * 128],
                        wo, start=True, stop=True,
                    )
                    out_sb = spool.tile([128, 512], F32, tag="outsb", bufs=8)
                    if last and qc % 2 == 1:
                        nc.scalar.copy(out_sb, out_ps)
                    else:
                        nc.vector.tensor_copy(out_sb, out_ps)
                    eng = nc.sync if qc % 2 == 0 else nc.gpsimd
                    eng.dma_start(
                        out=io["out_part"][rc * 128:(rc + 1) * 128, :],
                        in_=out_sb,
                    )

        # software-pipelined emission: later batches' proj and earlier
        # batches' epilogues fill engine gaps in the exp-paced attention
        for _it in range(loop_k):
            emit_proj(0)
            emit_attention(0, 0)
            emit_attention(0, 1)
            emit_proj(1)
            emit_attention(1, 0)
            emit_epilogue(0, last=False)
            emit_attention(1, 1)
            emit_epilogue(1, last=True)


def build_program(loop_k=1):
    nc = bacc.Bacc(
        "TRN2", target_bir_lowering=False, debug=False,
        enable_asserts=True, num_devices=NCORES,
    )
    io = {}
    for name, shape, dt_ in [
        ("xT", [D, R], F32R), ("wqkvT", [D, 192], F32R),
        ("woT", [64, D], BF16),
        ("cosw", [128, T], F32), ("sinw", [128, T], F32),
        ("consts1", [128, 384], F32R), ("ident", [128, 128], F32),
    ]:
        io[name] = nc.dram_tensor(name, shape, dt_, kind="ExternalInput").ap()
    io["out_part"] = nc.dram_tensor("out_part", [R, D], F32,
                                    kind="ExternalOutput").ap()
    with tile.TileContext(nc) as tc:
        _emit(tc, io, loop_k=loop_k)
    nc.compile()
    return nc


def host_constants():
    t = np.arange(T, dtype=np.float32)
    inv_freq = (1.0 / (10000.0 ** (np.arange(0, HD, 2, dtype=np.float32) / HD)))
    freqs = np.outer(t, inv_freq).astype(np.float32)      # (T, 16)
    emb = np.concatenate([freqs, freqs], axis=-1)         # (T, 32)
    cos = np.cos(emb).astype(np.float32)
    sin = np.sin(emb).astype(np.float32)
    cosw = np.tile(cos.T, (4, 1)).astype(np.float32)      # (128, 2048)
    ssin = sin.T.copy()
    ssin[:HD // 2] *= -1.0                                # signed sin
    sinw = np.tile(ssin, (4, 1)).astype(np.float32)

    permt = np.zeros((128, 128), dtype=np.float32)
    for blk in range(4):
        for m in range(HD):
            permt[blk * HD + (m + HD // 2) % HD, blk * HD + m] = 1.0

    a = np.arange(128)
    trilA = np.where(a[:, None] <= a[None, :], MASK_VAL, 0.0).astype(np.float32)
    trilB = np.where(a[:, None] > a[None, :], 1.0, 0.0).astype(np.float32)
    ident = np.eye(128, dtype=np.float32)
    consts1 = np.concatenate([permt, trilA, trilB], axis=1)
    return dict(cosw=cosw, sinw=sinw, ident=ident,
                consts1=np.ascontiguousarray(consts1))


def core_inputs(x, w_qkv, w_o):
    """Per-core input maps (core c owns heads 2c, 2c+1)."""
    x = np.asarray(x, dtype=np.float32)
    w_qkv = np.asarray(w_qkv, dtype=np.float32)
    w_o = np.asarray(w_o, dtype=np.float32)
    xT = np.ascontiguousarray(x.reshape(R, D).T)
    consts = host_constants()
    maps = []
    for c in range(NCORES):
        h0 = NHL * c
        qrows = w_qkv[h0 * HD:(h0 + NHL) * HD]                  # (64, 512)
        krows = w_qkv[D + h0 * HD:D + (h0 + NHL) * HD]
        vrows = w_qkv[2 * D + h0 * HD:2 * D + (h0 + NHL) * HD]
        m = dict(consts)
        m["xT"] = xT
        m["wqkvT"] = np.ascontiguousarray(
            np.concatenate([qrows, krows, vrows], axis=0).T)     # (512, 192)
        m["woT"] = np.ascontiguousarray(
            w_o[:, h0 * HD:(h0 + NHL) * HD].T).astype(ml_dtypes.bfloat16)
        maps.append(m)
    return maps


_PROG = None


def _get_prog():
    global _PROG
    if _PROG is None:
        _PROG = build_program()
    return _PROG


def kernel(x, w_qkv, w_o):
    nc = _get_prog()
    maps = core_inputs(x, w_qkv, w_o)
    res = run_bass_kernel_spmd(nc, maps, list(range(NCORES)))
    acc = np.zeros((R, D), dtype=np.float32)
    for i in range(NCORES):
        acc += res.results[i]["out_part"]
    return acc.reshape(B, T, D)



# revision 24
# speedup vs baseline: 55.7238x; 55.7238x over previous
"""Trainium2 Bass kernel for MultiHeadSelfAttention (RoPE + causal softmax).

Problem (hardcoded):
  x: (2, 2048, 512) f32, w_qkv: (1536, 512) f32, w_o: (512, 512) f32
  D_MODEL=512, N_HEADS=16, HEAD_DIM=32, ROPE_BASE=10000, causal.

Sharding: tensor-parallel over heads. Core c owns heads (2c, 2c+1) for both
batches; computes q/k/v projections from the full x, attention, and a
row-parallel partial of the output projection. The host sums the 8 partials.

v2 layout notes:
  - everything bf16 on the wire and in SBUF; PSUM accumulation stays f32.
  - v is projected directly in natural [row, feat] layout (contraction on
    the partition axis with xT chunks as lhsT), no transposes needed.
  - q,k produced transposed [feat, row], RoPE'd via block-diag permutation
    matmul + bf16 vector ops.
  - scores computed transposed (S.T [keys, queries]); causal mask added by
    a bf16 rank-128 triangular matmul into the same PSUM group.
  - exp on ACT (the bottleneck engine: it does nothing else), bf16 out.
  - attnout transposed back via XBAR dma_start_transpose (no PE/DVE work),
  - output projection partials DMA'd straight from PSUM (f32).
"""

import sys
import math
from contextlib import ExitStack

sys.path.insert(0, "/opt/trn_rl_repo")

import numpy as np
import ml_dtypes

import concourse.bass as bass
import concourse.tile as tile
from concourse import bacc, mybir
from concourse.bass_utils import run_bass_kernel_spmd

F32 = mybir.dt.float32
BF16 = mybir.dt.bfloat16
EXP = mybir.ActivationFunctionType.Exp

B = 2
T = 2048
D = 512
NH = 16
HD = 32
NCORES = 8
R = B * T            # 4096 rows, row = b*T + t
NHL = NH // NCORES   # 2 heads per core
KC = T // 128        # 16 key chunks per batch
SCALE = 1.0 / math.sqrt(HD)
MASK_VAL = -240.0


def _bcast_free(ap_2d, n_inner):
    """[P, n] -> [P, n, n_inner] AP with the inner dim broadcast (step 0)."""
    return bass.AP(
        tensor=ap_2d.tensor,
        offset=ap_2d.offset,
        ap=list(ap_2d.ap[:-1]) + [list(ap_2d.ap[-1]), [0, n_inner]],
    )


def _emit(tc, io, loop_k=1):
    nc = tc.nc
    with ExitStack() as ctx:
        cpool = ctx.enter_context(tc.tile_pool(name="consts", bufs=1))
        mpool = ctx.enter_context(tc.tile_pool(name="main", bufs=1))
        spool = ctx.enter_context(tc.tile_pool(name="small", bufs=3))
        ppool = ctx.enter_context(tc.tile_pool(name="pk", bufs=2))
        # PSUM budget (8 banks):
        #   tagA [128,1024] f32 x2 = 4 banks  (scores)
        #   tagB [128,512]  f32 x2 = 2 banks  (qk-proj / shift / v / out)
        #   tagC [128,8,33] f32 x2 = 2 banks  (av accumulator groups)
        psum = ctx.enter_context(tc.tile_pool(name="psum", bufs=1, space="PSUM"))

        def tile_a():
            return psum.tile([128, 1024], F32, tag="A", bufs=2, name="psA")

        def tile_b(p=128, w=512):
            return psum.tile([p, w], F32, tag="B", bufs=2, name="psB")

        def tile_c():
            return psum.tile([128, 4, HD + 1], F32, tag="C", bufs=2, name="psC")

        # ---- constants (batched DMAs, spread over issue queues; the ACT
        # queue is idle at start so it carries the rope tables) ----
        cmix = cpool.tile([128, 512], BF16, tag="cmix")
        nc.gpsimd.dma_start(out=cmix, in_=io["consts1"])
        permt = cmix[:, 0:128]
        trilA = cmix[:, 128:256]
        trilB = cmix[:, 256:384]
        identb = cmix[:, 384:512]
        wqkv = []
        for dc in range(4):
            w_t = cpool.tile([128, 192], BF16, tag=f"wqkv{dc}")
            nc.gpsimd.dma_start(out=w_t, in_=io["wqkvT"][dc * 128:(dc + 1) * 128, :])
            wqkv.append(w_t)
        wo = cpool.tile([128, 512], BF16, tag="wo")
        nc.gpsimd.dma_start(out=wo[0:64, :], in_=io["woT"])
        nc.gpsimd.dma_start(out=wo[64:128, :], in_=io["woT"])

        # ---- persistent activations ----
        qkr = mpool.tile([128, R], BF16, tag="qkr")          # RoPE'd qT/kT
        ka = mpool.tile([64, R], BF16, tag="ka")             # k-half, base-aligned
        vall = mpool.tile([128, R // 128, NHL, HD + 1], BF16, tag="vall")
        # attnout natural: query-chunk PAIRS share a 128-wide slot so the
        # XBAR transpose moves no padding; aoT holds both heads' features of
        # the even chunk on partitions 0-63 and of the odd chunk on 64-127
        ao = mpool.tile([128, B, KC, 2, HD], BF16, tag="ao")
        aoT = mpool.tile([128, R // 2], BF16, tag="aoT")
        cosw = mpool.tile([128, T], BF16, tag="cosw")       # one batch (shared)
        sinw = mpool.tile([128, T], BF16, tag="sinw")

        warm = cpool.tile([128, 2], F32, tag="warm")
        nc.vector.memset(warm[:, 0:1], 0.0)
        nc.scalar.activation(out=warm[:, 1:2], in_=warm[:, 0:1], func=EXP)
        nc.vector.memset(vall[:, :, :, HD:HD + 1], 1.0)     # softmax-sum column

        def emit_xt(bb, prefetch=False):
            xt = [mpool.tile([128, T], BF16, tag=f"xt{dc}", bufs=2,
                             name=f"xt{dc}") for dc in range(4)]
            for j in range(4):
                for dc in range(4):
                    if prefetch:
                        eng = nc.sync
                    else:
                        eng = nc.sync if j < 3 else nc.gpsimd
                    eng.dma_start(
                        out=xt[dc][:, j * 512:(j + 1) * 512],
                        in_=io["xT"][dc * 128:(dc + 1) * 128,
                                     bb * T + j * 512:bb * T + (j + 1) * 512],
                    )
            return xt

        def emit_proj_jl(bb, xt, jl):
            colb = slice(jl * 512, (jl + 1) * 512)          # batch-local
            cols = slice(bb * T + jl * 512, bb * T + (jl + 1) * 512)
            # qT/kT projection: [feat, row] = wqkT.T @ xT
            qk_ps = tile_b()
            for dc in range(4):
                nc.tensor.matmul(
                    qk_ps, wqkv[dc][:, 0:128], xt[dc][:, colb],
                    start=(dc == 0), stop=(dc == 3),
                )
            # rotate_half via block-diag permutation (needs SBUF copy)
            qks = spool.tile([128, 512], BF16, tag="qks")
            nc.vector.tensor_copy(qks, qk_ps)
            # cos-term from the bf16 copy (SBUF->SBUF: legal on gpsimd)
            nc.gpsimd.tensor_mul(qkr[:, cols], qks, cosw[:, colb])
            # v projection directly in natural [row, feat] layout (PE filler
            # while the qks copy completes)
            v_ps = psum.tile([128, 4, 64], F32, tag="B", bufs=2,
                             name="psBv")
            for rr in range(4):
                rsl = slice(jl * 512 + rr * 128, jl * 512 + rr * 128 + 128)
                for dc in range(4):
                    nc.tensor.matmul(
                        v_ps[:, rr, :],
                        xt[dc][:, rsl], wqkv[dc][:, 128:192],
                        start=(dc == 0), stop=(dc == 3),
                        skip_group_check=True,
                    )
            sh_ps = tile_b()
            nc.tensor.matmul(sh_ps, permt, qks, start=True, stop=True)
            # qkr += shifted*sin_signed
            t1 = spool.tile([128, 512], BF16, tag="t1")
            nc.vector.tensor_mul(t1, sh_ps, sinw[:, colb])
            nc.vector.tensor_add(qkr[:, cols], qkr[:, cols], t1)
            # partition-aligned copy of the k rows (matmul requires lhsT
            # and rhs to share a base partition)
            nc.gpsimd.tensor_copy(ka[:, cols], qkr[64:128, cols])
            rc0 = bb * KC + jl * 4
            for hh in range(NHL):
                nc.vector.tensor_copy(
                    vall[:, rc0:rc0 + 4, hh, 0:HD],
                    v_ps[:, :, hh * HD:(hh + 1) * HD],
                )

        def emit_proj(bb, xt):
            for jl in range(4):
                emit_proj_jl(bb, xt, jl)

        def emit_attention(bb, hh, epi=None, first=False, projnext=None,
                           tailprev=None):
            qrow = 32 * hh            # q rows in qkr
            krow = 32 * hh            # k rows in ka
            ppks = []
            pavs = {}

            def av_column(qc):
                # av column for qc (P rows kc<=qc all exist);
                # 4 query chunks per PSUM group, normalized per group
                g = qc // 4
                if qc % 4 == 0:
                    pavs[g] = tile_c()
                slot = pavs[g][:, qc % 4, :]
                for kp in range(qc + 1):
                    nc.tensor.matmul(
                        slot,
                        ppks[kp][:, 128 * (qc - kp):128 * (qc - kp) + 128],
                        vall[:, bb * KC + kp, hh, :],
                        start=(kp == 0), stop=(kp == qc),
                    )
                if qc % 4 == 3:
                    # normalize this group: attnout = av / l
                    pav = pavs[g]
                    rl = spool.tile([128, 4, 1], F32, tag="rl")
                    nc.vector.reciprocal(rl, pav[:, :, HD:HD + 1])
                    nc.vector.tensor_mul(
                        ao[:, bb, g * 4:(g + 1) * 4, hh, :],
                        pav[:, :, 0:HD],
                        _bcast_free(rl[:, :, 0], HD),
                    )

            # av columns trail the score/exp stream by 2 key chunks so
            # the PE never stalls waiting for the exp it just queued
            for kc in range(KC + 2):
                if kc >= 2:
                    av_column(kc - 2)
                if kc < KC:
                    n_kc = T - 128 * kc
                    cw = 512 if (first and kc == 0) else 1024
                    kslc = slice(bb * T + 128 * kc, bb * T + 128 * (kc + 1))
                    ppk = ppool.tile([128, n_kc], BF16, tag=f"ppk{kc}",
                                     bufs=(2 if kc < 5 else 1),
                                     name=f"ppk{kc}")
                    ppks.append(ppk)
                    for c0 in range(0, n_kc, cw):
                        nt = min(cw, n_kc - c0)
                        sc_ps = tile_a()
                        for c in range(c0, c0 + nt, 512):
                            ln = min(512, n_kc - c)
                            qslc = slice(bb * T + 128 * kc + c,
                                         bb * T + 128 * kc + c + ln)
                            nc.tensor.matmul(
                                sc_ps[:, c - c0:c - c0 + ln],
                                ka[krow:krow + 32, kslc],
                                qkr[qrow:qrow + 32, qslc],
                                start=True, stop=(c > 0),
                                skip_group_check=True,
                            )
                        if c0 == 0:
                            # causal mask on the diagonal 128x128 block:
                            # accumulates -240*max(0, k-q)
                            nc.tensor.matmul(
                                sc_ps[:, 0:128], trilA, trilB,
                                start=False, stop=True,
                                skip_group_check=True,
                            )
                        nc.scalar.activation(
                            out=ppk[:, c0:c0 + nt],
                            in_=sc_ps[:, 0:nt],
                            func=EXP, scale=SCALE,
                        )
                if tailprev is not None and kc == 2:
                    emit_epi_tail(tailprev)
                # interleave next batch's projection into this unit's slack
                if projnext is not None and kc in (6, 9, 12, 15):
                    emit_proj_jl(projnext[0], projnext[1], (kc - 6) // 3)
                # epilogue group g of this batch is ready once both heads'
                # group-g attnout is normalized (kc == 4g+5 in this loop);
                # its PE work is deferred 2 iterations so the transpose
                # roundtrip never head-of-line blocks the PE queue
                lastu = (bb == 1 and hh == 1)
                if epi is not None and kc >= 5 and (kc - 5) % 4 == 0 \
                        and (kc - 5) // 4 <= (1 if lastu else 2):
                    emit_epi_transposes(epi, (kc - 5) // 4)
                if epi is not None and kc >= 7 and (kc - 7) % 4 == 0 \
                        and (kc - 7) // 4 <= (1 if lastu else 2):
                    emit_epi_proj(epi, (kc - 7) // 4)
                if epi is not None and lastu and kc == 15:
                    emit_epi_pe_group(epi, 2)

        def emit_epi_transposes(bb, g):
            # XBAR-transpose attnout for 2 query-chunk pairs
            for jj in range(2):
                pr = g * 2 + jj
                pc = bb * (KC // 2) + pr
                nc.sync.dma_start_transpose(
                    aoT[:, pc * 128:(pc + 1) * 128],
                    ao[:, bb, 2 * pr:2 * pr + 2, :, :]
                    .rearrange("p a b c -> p (a b c)"),
                )

        def emit_epi_proj(bb, g, tail=False, act_ok=False):
            for qc in range(g * 4, g * 4 + 4):
                rc = bb * KC + qc
                if tail:
                    # scores banks are free at the tail: use them to avoid
                    # the B-buffer rotation serializing the drain
                    pa = tile_a()
                    out_ps = pa[:, 0:512] if qc % 2 == 0 else pa[:, 512:1024]
                else:
                    out_ps = tile_b()
                pc = bb * (KC // 2) + qc // 2
                hb = (qc % 2) * 64
                nc.tensor.matmul(
                    out_ps,
                    aoT[hb:hb + 64, pc * 128:(pc + 1) * 128],
                    wo[hb:hb + 64, :], start=True, stop=True,
                    skip_group_check=True,
                )
                out_sb = spool.tile([128, 512], BF16, tag="outsb", bufs=8)
                if tail and act_ok:
                    # ACT is drained at the tail: press it into service
                    if qc % 2 == 0:
                        nc.scalar.activation(
                            out=out_sb, in_=out_ps,
                            func=mybir.ActivationFunctionType.Copy)
                        eng = nc.scalar
                    else:
                        nc.vector.tensor_copy(out_sb, out_ps)
                        eng = nc.sync if qc % 4 == 1 else nc.gpsimd
                else:
                    nc.vector.tensor_copy(out_sb, out_ps)
                    eng = nc.sync if qc % 2 == 0 else nc.gpsimd
                eng.dma_start(
                    out=io["out_part"][rc * 128:(rc + 1) * 128, :],
                    in_=out_sb,
                )

        def emit_epi_pe_group(bb, g, act_ok=False):
            # PE-transpose route: skips the XBAR DMA-completion semaphore
            # latency (only worth it when the stream is ending)
            at_ps = psum.tile([128, 256], BF16, tag="A", bufs=2, name="psAt")
            for jj in range(2):
                pr = 2 * g + jj
                nc.tensor.transpose(
                    at_ps[:, jj * 128:(jj + 1) * 128],
                    ao[:, bb, 2 * pr:2 * pr + 2, :, :]
                    .rearrange("p a b c -> p (a b c)"),
                    identb,
                )
            pc = bb * (KC // 2) + 2 * g
            nc.vector.tensor_copy(aoT[:, pc * 128:pc * 128 + 256], at_ps)
            emit_epi_proj(bb, g, tail=act_ok, act_ok=act_ok)

        def emit_epi_tail(bb, act_ok=False):
            if not act_ok:
                # mid-stream: XBAR route, keep the score banks out of it
                emit_epi_transposes(bb, 3)
                emit_epi_proj(bb, 3)
                return
            emit_epi_pe_group(bb, 3, act_ok=True)

        # software-pipelined emission: later batches' proj and earlier
        # batches' epilogues fill engine gaps in the exp-paced attention
        nc.scalar.dma_start(out=cosw[:, 0:1024], in_=io["cosw"][:, 0:1024])
        nc.scalar.dma_start(out=sinw[:, 0:1024], in_=io["sinw"][:, 0:1024])
        nc.sync.dma_start(out=cosw[:, 1024:T], in_=io["cosw"][:, 1024:T])
        nc.sync.dma_start(out=sinw[:, 1024:T], in_=io["sinw"][:, 1024:T])
        for _it in range(loop_k):
            xt0 = emit_xt(0)
            emit_proj(0, xt0)
            xt1 = emit_xt(1, prefetch=True)
            emit_attention(0, 0, first=True, projnext=(1, xt1))
            emit_attention(0, 1, epi=0)
            emit_attention(1, 0, tailprev=0)
            emit_attention(1, 1, epi=1)
            emit_epi_tail(1, act_ok=True)


def build_program(loop_k=1):
    nc = bacc.Bacc(
        "TRN2", target_bir_lowering=False, debug=False,
        enable_asserts=True, num_devices=NCORES,
    )
    io = {}
    for name, shape, dt_ in [
        ("xT", [D, R], BF16), ("wqkvT", [D, 192], BF16),
        ("woT", [64, D], BF16),
        ("cosw", [128, T], BF16), ("sinw", [128, T], BF16),
        ("consts1", [128, 512], BF16),
    ]:
        io[name] = nc.dram_tensor(name, shape, dt_, kind="ExternalInput").ap()
    io["out_part"] = nc.dram_tensor("out_part", [R, D], BF16,
                                    kind="ExternalOutput").ap()
    with tile.TileContext(nc) as tc:
        _emit(tc, io, loop_k=loop_k)
    nc.compile()
    return nc


def host_constants():
    t = np.arange(T, dtype=np.float32)
    inv_freq = (1.0 / (10000.0 ** (np.arange(0, HD, 2, dtype=np.float32) / HD)))
    freqs = np.outer(t, inv_freq).astype(np.float32)      # (T, 16)
    emb = np.concatenate([freqs, freqs], axis=-1)         # (T, 32)
    cos = np.cos(emb).astype(np.float32)
    sin = np.sin(emb).astype(np.float32)
    cosw = np.tile(cos.T, (4, 1)).astype(ml_dtypes.bfloat16)   # (128, 2048)
    ssin = sin.T.copy()
    ssin[:HD // 2] *= -1.0                                # signed sin
    sinw = np.tile(ssin, (4, 1)).astype(ml_dtypes.bfloat16)

    permt = np.zeros((128, 128), dtype=np.float32)
    for blk in range(4):
        for m in range(HD):
            permt[blk * HD + (m + HD // 2) % HD, blk * HD + m] = 1.0

    a = np.arange(128)
    trilA = np.where(a[:, None] <= a[None, :], MASK_VAL, 0.0).astype(np.float32)
    trilB = np.where(a[:, None] > a[None, :], 1.0, 0.0).astype(np.float32)
    consts1 = np.concatenate([permt, trilA, trilB, np.eye(128, dtype=np.float32)],
                             axis=1)
    return dict(cosw=cosw, sinw=sinw,
                consts1=np.ascontiguousarray(consts1).astype(ml_dtypes.bfloat16))


def core_inputs(x, w_qkv, w_o):
    """Per-core input maps (core c owns heads 2c, 2c+1)."""
    x = np.asarray(x, dtype=np.float32)
    w_qkv = np.asarray(w_qkv, dtype=np.float32)
    w_o = np.asarray(w_o, dtype=np.float32)
    xT = np.ascontiguousarray(x.reshape(R, D).T).astype(ml_dtypes.bfloat16)
    consts = host_constants()
    maps = []
    for c in range(NCORES):
        h0 = NHL * c
        qrows = w_qkv[h0 * HD:(h0 + NHL) * HD]                  # (64, 512)
        krows = w_qkv[D + h0 * HD:D + (h0 + NHL) * HD]
        vrows = w_qkv[2 * D + h0 * HD:2 * D + (h0 + NHL) * HD]
        m = dict(consts)
        m["xT"] = xT
        m["wqkvT"] = np.ascontiguousarray(
            np.concatenate([qrows, krows, vrows], axis=0).T
        ).astype(ml_dtypes.bfloat16)                             # (512, 192)
        m["woT"] = np.ascontiguousarray(
            w_o[:, h0 * HD:(h0 + NHL) * HD].T).astype(ml_dtypes.bfloat16)
        maps.append(m)
    return maps


_PROG = None


def _get_prog():
    global _PROG
    if _PROG is None:
        _PROG = build_program()
    return _PROG


def kernel(x, w_qkv, w_o):
    nc = _get_prog()
    maps = core_inputs(x, w_qkv, w_o)
    res = run_bass_kernel_spmd(nc, maps, list(range(NCORES)))
    acc = np.zeros((R, D), dtype=np.float32)
    for i in range(NCORES):
        acc += res.results[i]["out_part"].astype(np.float32)
    return acc.reshape(B, T, D)


# revision 30
# speedup vs baseline: 71.6773x; 1.2863x over previous
"""Trainium2 Bass kernel for MultiHeadSelfAttention (RoPE + causal softmax).

Problem (hardcoded):
  x: (2, 2048, 512) f32, w_qkv: (1536, 512) f32, w_o: (512, 512) f32
  D_MODEL=512, N_HEADS=16, HEAD_DIM=32, ROPE_BASE=10000, causal.

Sharding: tensor-parallel over heads. Core c owns heads (2c, 2c+1) for both
batches; computes q/k/v projections from the full x, attention, and a
row-parallel partial of the output projection. The host sums the 8 partials.

v2 layout notes:
  - everything bf16 on the wire and in SBUF; PSUM accumulation stays f32.
  - v is projected directly in natural [row, feat] layout (contraction on
    the partition axis with xT chunks as lhsT), no transposes needed.
  - q,k produced transposed [feat, row], RoPE'd via block-diag permutation
    matmul + bf16 vector ops.
  - scores computed transposed (S.T [keys, queries]); causal mask added by
    a bf16 rank-128 triangular matmul into the same PSUM group.
  - exp on ACT (the bottleneck engine: it does nothing else), bf16 out.
  - attnout transposed back via XBAR dma_start_transpose (no PE/DVE work),
  - output projection partials DMA'd straight from PSUM (f32).
"""

import sys
import math
from contextlib import ExitStack

sys.path.insert(0, "/opt/trn_rl_repo")

import numpy as np
import ml_dtypes

import concourse.bass as bass
import concourse.tile as tile
from concourse import bacc, mybir
from concourse.bass_utils import run_bass_kernel_spmd

F32 = mybir.dt.float32
BF16 = mybir.dt.bfloat16
EXP = mybir.ActivationFunctionType.Exp

B = 2
T = 2048
D = 512
NH = 16
HD = 32
NCORES = 8
R = B * T            # 4096 rows, row = b*T + t
NHL = NH // NCORES   # 2 heads per core
KC = T // 128        # 16 key chunks per batch
SCALE = 1.0 / math.sqrt(HD)
MASK_VAL = -240.0


def _bcast_free(ap_2d, n_inner):
    """[P, n] -> [P, n, n_inner] AP with the inner dim broadcast (step 0)."""
    return bass.AP(
        tensor=ap_2d.tensor,
        offset=ap_2d.offset,
        ap=list(ap_2d.ap[:-1]) + [list(ap_2d.ap[-1]), [0, n_inner]],
    )


def _emit(tc, io, loop_k=1):
    nc = tc.nc
    with ExitStack() as ctx:
        cpool = ctx.enter_context(tc.tile_pool(name="consts", bufs=1))
        mpool = ctx.enter_context(tc.tile_pool(name="main", bufs=1))
        spool = ctx.enter_context(tc.tile_pool(name="small", bufs=3))
        ppool = ctx.enter_context(tc.tile_pool(name="pk", bufs=2))
        # PSUM budget (8 banks):
        #   tagA [128,1024] f32 x2 = 4 banks  (scores)
        #   tagB [128,512]  f32 x2 = 2 banks  (qk-proj / shift / v / out)
        #   tagC [128,8,33] f32 x2 = 2 banks  (av accumulator groups)
        psum = ctx.enter_context(tc.tile_pool(name="psum", bufs=1, space="PSUM"))

        def tile_a():
            return psum.tile([128, 1024], F32, tag="A", bufs=2, name="psA")

        def tile_b(p=128, w=512):
            return psum.tile([p, w], F32, tag="B", bufs=2, name="psB")

        def tile_c():
            return psum.tile([128, 4, HD + 1], F32, tag="C", bufs=2, name="psC")

        # ---- constants (batched DMAs, spread over issue queues; the ACT
        # queue is idle at start so it carries the rope tables) ----
        cmix = cpool.tile([128, 512], BF16, tag="cmix")
        nc.gpsimd.dma_start(out=cmix, in_=io["consts1"])
        permt = cmix[:, 0:128]
        trilA = cmix[:, 128:256]
        trilB = cmix[:, 256:384]
        identb = cmix[:, 384:512]
        wqkv = []
        for dc in range(4):
            w_t = cpool.tile([128, 192], BF16, tag=f"wqkv{dc}")
            nc.gpsimd.dma_start(out=w_t, in_=io["wqkvT"][dc * 128:(dc + 1) * 128, :])
            wqkv.append(w_t)
        wo = cpool.tile([128, 512], BF16, tag="wo")
        nc.gpsimd.dma_start(out=wo[0:64, :], in_=io["woT"])
        nc.gpsimd.dma_start(out=wo[64:128, :], in_=io["woT"])

        # ---- persistent activations ----
        qkr = mpool.tile([128, R], BF16, tag="qkr")          # RoPE'd qT/kT
        ka = mpool.tile([64, R], BF16, tag="ka")             # k-half, base-aligned
        vall = mpool.tile([128, R // 128, NHL, HD + 1], BF16, tag="vall")
        # attnout natural: query-chunk PAIRS share a 128-wide slot so the
        # XBAR transpose moves no padding; aoT holds both heads' features of
        # the even chunk on partitions 0-63 and of the odd chunk on 64-127
        ao = mpool.tile([128, B, KC, 2, HD], BF16, tag="ao")
        aoT = mpool.tile([128, R // 2], BF16, tag="aoT")
        cosw = mpool.tile([128, T], BF16, tag="cosw")       # one batch (shared)
        sinw = mpool.tile([128, T], BF16, tag="sinw")

        warm = cpool.tile([128, 2], F32, tag="warm")
        nc.vector.memset(warm[:, 0:1], 0.0)
        nc.scalar.activation(out=warm[:, 1:2], in_=warm[:, 0:1], func=EXP)
        nc.vector.memset(vall[:, :, :, HD:HD + 1], 1.0)     # softmax-sum column

        def emit_xt(bb, prefetch=False):
            xt = [mpool.tile([128, T], BF16, tag=f"xt{dc}", bufs=2,
                             name=f"xt{dc}") for dc in range(4)]
            for j in range(4):
                for dc in range(4):
                    if prefetch:
                        eng = nc.sync
                    else:
                        eng = nc.sync if j < 3 else nc.gpsimd
                    eng.dma_start(
                        out=xt[dc][:, j * 512:(j + 1) * 512],
                        in_=io["xT"][dc * 128:(dc + 1) * 128,
                                     bb * T + j * 512:bb * T + (j + 1) * 512],
                    )
            return xt

        def emit_proj_jl(bb, xt, jl):
            colb = slice(jl * 512, (jl + 1) * 512)          # batch-local
            cols = slice(bb * T + jl * 512, bb * T + (jl + 1) * 512)
            # qT/kT projection: [feat, row] = wqkT.T @ xT
            qk_ps = tile_b()
            for dc in range(4):
                nc.tensor.matmul(
                    qk_ps, wqkv[dc][:, 0:128], xt[dc][:, colb],
                    start=(dc == 0), stop=(dc == 3),
                )
            # rotate_half via block-diag permutation (needs SBUF copy)
            qks = spool.tile([128, 512], BF16, tag="qks")
            nc.vector.tensor_copy(qks, qk_ps)
            # cos-term from the bf16 copy (SBUF->SBUF: legal on gpsimd)
            nc.gpsimd.tensor_mul(qkr[:, cols], qks, cosw[:, colb])
            # v projection directly in natural [row, feat] layout (PE filler
            # while the qks copy completes)
            v_ps = psum.tile([128, 4, 64], F32, tag="B", bufs=2,
                             name="psBv")
            for rr in range(4):
                rsl = slice(jl * 512 + rr * 128, jl * 512 + rr * 128 + 128)
                for dc in range(4):
                    nc.tensor.matmul(
                        v_ps[:, rr, :],
                        xt[dc][:, rsl], wqkv[dc][:, 128:192],
                        start=(dc == 0), stop=(dc == 3),
                        skip_group_check=True,
                    )
            sh_ps = tile_b()
            nc.tensor.matmul(sh_ps, permt, qks, start=True, stop=True)
            # qkr += shifted*sin_signed
            t1 = spool.tile([128, 512], BF16, tag="t1")
            nc.vector.tensor_mul(t1, sh_ps, sinw[:, colb])
            nc.vector.tensor_add(qkr[:, cols], qkr[:, cols], t1)
            # partition-aligned copy of the k rows (matmul requires lhsT
            # and rhs to share a base partition)
            nc.gpsimd.tensor_copy(ka[:, cols], qkr[64:128, cols])
            rc0 = bb * KC + jl * 4
            for hh in range(NHL):
                nc.vector.tensor_copy(
                    vall[:, rc0:rc0 + 4, hh, 0:HD],
                    v_ps[:, :, hh * HD:(hh + 1) * HD],
                )

        def emit_proj(bb, xt):
            for jl in range(4):
                emit_proj_jl(bb, xt, jl)

        def emit_attention(bb, hh, epi=None, first=False, projnext=None,
                           tailprev=None, hooks=None):
            qrow = 32 * hh            # q rows in qkr
            krow = 32 * hh            # k rows in ka
            ppks = []
            pavs = {}

            def av_column(qc):
                # av column for qc (P rows kc<=qc all exist);
                # 4 query chunks per PSUM group, normalized per group
                g = qc // 4
                if qc % 4 == 0:
                    pavs[g] = tile_c()
                slot = pavs[g][:, qc % 4, :]
                for kp in range(qc + 1):
                    nc.tensor.matmul(
                        slot,
                        ppks[kp][:, 128 * (qc - kp):128 * (qc - kp) + 128],
                        vall[:, bb * KC + kp, hh, :],
                        start=(kp == 0), stop=(kp == qc),
                    )
                if qc % 4 == 3:
                    # normalize this group: attnout = av / l
                    pav = pavs[g]
                    rl = spool.tile([128, 4, 1], F32, tag="rl")
                    nc.vector.reciprocal(rl, pav[:, :, HD:HD + 1])
                    nc.vector.tensor_mul(
                        ao[:, bb, g * 4:(g + 1) * 4, hh, :],
                        pav[:, :, 0:HD],
                        _bcast_free(rl[:, :, 0], HD),
                    )

            # av columns trail the score/exp stream by 2 key chunks so
            # the PE never stalls waiting for the exp it just queued
            for kc in range(KC + 2):
                if kc >= 2:
                    av_column(kc - 2)
                if kc < KC:
                    n_kc = T - 128 * kc
                    cw = 512 if (first and kc == 0) else 1024
                    kslc = slice(bb * T + 128 * kc, bb * T + 128 * (kc + 1))
                    ppk = ppool.tile([128, n_kc], BF16, tag=f"ppk{kc}",
                                     bufs=2, name=f"ppk{kc}")
                    ppks.append(ppk)
                    for c0 in range(0, n_kc, cw):
                        nt = min(cw, n_kc - c0)
                        sc_ps = tile_a()
                        for c in range(c0, c0 + nt, 512):
                            ln = min(512, n_kc - c)
                            qslc = slice(bb * T + 128 * kc + c,
                                         bb * T + 128 * kc + c + ln)
                            nc.tensor.matmul(
                                sc_ps[:, c - c0:c - c0 + ln],
                                ka[krow:krow + 32, kslc],
                                qkr[qrow:qrow + 32, qslc],
                                start=True, stop=(c > 0),
                                skip_group_check=True,
                            )
                        if c0 == 0:
                            # causal mask on the diagonal 128x128 block:
                            # accumulates -240*max(0, k-q)
                            nc.tensor.matmul(
                                sc_ps[:, 0:128], trilA, trilB,
                                start=False, stop=True,
                                skip_group_check=True,
                            )
                        nc.scalar.activation(
                            out=ppk[:, c0:c0 + nt],
                            in_=sc_ps[:, 0:nt],
                            func=EXP, scale=SCALE,
                        )
                # interleave next batch's projection into this unit's slack
                if projnext is not None and kc in (6, 9, 12, 15):
                    emit_proj_jl(projnext[0], projnext[1], (kc - 6) // 3)
                # epilogue/filler hooks (deps resolved well before the hook
                # point so the PE queue never head-of-line blocks)
                if hooks and kc in hooks:
                    for fn in hooks[kc]:
                        fn()

        def emit_epi_transposes(bb, g):
            # XBAR-transpose attnout for 2 query-chunk pairs
            for jj in range(2):
                pr = g * 2 + jj
                pc = bb * (KC // 2) + pr
                nc.sync.dma_start_transpose(
                    aoT[:, pc * 128:(pc + 1) * 128],
                    ao[:, bb, 2 * pr:2 * pr + 2, :, :]
                    .rearrange("p a b c -> p (a b c)"),
                )

        def emit_epi_proj(bb, g, tail=False, act_ok=False):
            for qc in range(g * 4, g * 4 + 4):
                rc = bb * KC + qc
                if tail:
                    # scores banks are free at the tail: use them to avoid
                    # the B-buffer rotation serializing the drain
                    pa = tile_a()
                    out_ps = pa[:, 0:512] if qc % 2 == 0 else pa[:, 512:1024]
                else:
                    out_ps = tile_b()
                pc = bb * (KC // 2) + qc // 2
                hb = (qc % 2) * 64
                nc.tensor.matmul(
                    out_ps,
                    aoT[hb:hb + 64, pc * 128:(pc + 1) * 128],
                    wo[hb:hb + 64, :], start=True, stop=True,
                    skip_group_check=True,
                )
                out_sb = spool.tile([128, 512], BF16, tag="outsb", bufs=8)
                if tail and act_ok:
                    # ACT is drained at the tail: press it into service
                    if qc % 2 == 0:
                        nc.scalar.activation(
                            out=out_sb, in_=out_ps,
                            func=mybir.ActivationFunctionType.Copy)
                        eng = nc.scalar
                    else:
                        nc.vector.tensor_copy(out_sb, out_ps)
                        eng = nc.sync if qc % 4 == 1 else nc.gpsimd
                else:
                    nc.vector.tensor_copy(out_sb, out_ps)
                    eng = nc.sync if qc % 2 == 0 else nc.gpsimd
                eng.dma_start(
                    out=io["out_part"][rc * 128:(rc + 1) * 128, :],
                    in_=out_sb,
                )

        def emit_epi_pe_group(bb, g, act_ok=False):
            # PE-transpose route: skips the XBAR DMA-completion semaphore
            # latency (only worth it when the stream is ending)
            at_ps = psum.tile([128, 256], BF16, tag="A", bufs=2, name="psAt")
            for jj in range(2):
                pr = 2 * g + jj
                nc.tensor.transpose(
                    at_ps[:, jj * 128:(jj + 1) * 128],
                    ao[:, bb, 2 * pr:2 * pr + 2, :, :]
                    .rearrange("p a b c -> p (a b c)"),
                    identb,
                )
            pc = bb * (KC // 2) + 2 * g
            nc.vector.tensor_copy(aoT[:, pc * 128:pc * 128 + 256], at_ps)
            emit_epi_proj(bb, g, tail=act_ok, act_ok=act_ok)

        def emit_epi_tail(bb, act_ok=False):
            if not act_ok:
                # mid-stream: XBAR route, keep the score banks out of it
                emit_epi_transposes(bb, 3)
                emit_epi_proj(bb, 3)
                return
            emit_epi_pe_group(bb, 3, act_ok=True)

        # software-pipelined emission: later batches' proj and earlier
        # batches' epilogues fill engine gaps in the exp-paced attention
        nc.scalar.dma_start(out=cosw[:, 0:1024], in_=io["cosw"][:, 0:1024])
        nc.scalar.dma_start(out=sinw[:, 0:1024], in_=io["sinw"][:, 0:1024])
        # iteration-0 prologue; later iterations' batch-0 projections are
        # software-pipelined into the PREVIOUS iteration's last unit
        xt0 = emit_xt(0)
        nc.sync.dma_start(out=cosw[:, 1024:T], in_=io["cosw"][:, 1024:T])
        nc.sync.dma_start(out=sinw[:, 1024:T], in_=io["sinw"][:, 1024:T])
        emit_proj(0, xt0)
        for _it in range(loop_k):
            last_it = (_it == loop_k - 1)
            xt1 = emit_xt(1, prefetch=True)
            emit_attention(0, 0, first=(_it == 0), projnext=(1, xt1))
            emit_attention(0, 1, hooks={
                5: [lambda: emit_epi_transposes(0, 0)],
                7: [lambda: emit_epi_proj(0, 0)],
                11: [lambda: emit_epi_transposes(0, 1)],
                13: [lambda: emit_epi_proj(0, 1)],
            })
            if not last_it:
                xt0 = emit_xt(0)
            emit_attention(1, 0, hooks={
                3: [lambda: emit_epi_transposes(0, 2)],
                5: [lambda: emit_epi_proj(0, 2)],
                8: [lambda: emit_epi_tail(0)],
            })
            hooks_11 = {
                5: [lambda: emit_epi_transposes(1, 0)],
                7: [lambda: emit_epi_proj(1, 0)],
                11: [lambda: emit_epi_transposes(1, 1)],
                13: [lambda: emit_epi_proj(1, 1)],
            }
            if last_it:
                hooks_11[15] = [lambda: emit_epi_pe_group(1, 2, act_ok=True)]
            else:
                # next iteration's batch-0 projection rides in this unit
                hooks_11[6] = [lambda: emit_proj_jl(0, xt0, 0)]
                hooks_11[9] = [lambda: emit_proj_jl(0, xt0, 1)]
                hooks_11[12] = [lambda: emit_proj_jl(0, xt0, 2)]
                hooks_11[15] = [lambda: emit_proj_jl(0, xt0, 3),
                                lambda: emit_epi_transposes(1, 2)]
            emit_attention(1, 1, hooks=hooks_11)
            if last_it:
                emit_epi_tail(1, act_ok=True)
            else:
                emit_epi_proj(1, 2)
                emit_epi_tail(1)


def build_program(loop_k=1):
    nc = bacc.Bacc(
        "TRN2", target_bir_lowering=False, debug=False,
        enable_asserts=True, num_devices=NCORES,
    )
    io = {}
    for name, shape, dt_ in [
        ("xT", [D, R], BF16), ("wqkvT", [D, 192], BF16),
        ("woT", [64, D], BF16),
        ("cosw", [128, T], BF16), ("sinw", [128, T], BF16),
        ("consts1", [128, 512], BF16),
    ]:
        io[name] = nc.dram_tensor(name, shape, dt_, kind="ExternalInput").ap()
    io["out_part"] = nc.dram_tensor("out_part", [R, D], BF16,
                                    kind="ExternalOutput").ap()
    with tile.TileContext(nc) as tc:
        _emit(tc, io, loop_k=loop_k)
    nc.compile()
    return nc


def host_constants():
    t = np.arange(T, dtype=np.float32)
    inv_freq = (1.0 / (10000.0 ** (np.arange(0, HD, 2, dtype=np.float32) / HD)))
    freqs = np.outer(t, inv_freq).astype(np.float32)      # (T, 16)
    emb = np.concatenate([freqs, freqs], axis=-1)         # (T, 32)
    cos = np.cos(emb).astype(np.float32)
    sin = np.sin(emb).astype(np.float32)
    cosw = np.tile(cos.T, (4, 1)).astype(ml_dtypes.bfloat16)   # (128, 2048)
    ssin = sin.T.copy()
    ssin[:HD // 2] *= -1.0                                # signed sin
    sinw = np.tile(ssin, (4, 1)).astype(ml_dtypes.bfloat16)

    permt = np.zeros((128, 128), dtype=np.float32)
    for blk in range(4):
        for m in range(HD):
            permt[blk * HD + (m + HD // 2) % HD, blk * HD + m] = 1.0

    a = np.arange(128)
    trilA = np.where(a[:, None] <= a[None, :], MASK_VAL, 0.0).astype(np.float32)
    trilB = np.where(a[:, None] > a[None, :], 1.0, 0.0).astype(np.float32)
    consts1 = np.concatenate([permt, trilA, trilB, np.eye(128, dtype=np.float32)],
                             axis=1)
    return dict(cosw=cosw, sinw=sinw,
                consts1=np.ascontiguousarray(consts1).astype(ml_dtypes.bfloat16))


def core_inputs(x, w_qkv, w_o):
    """Per-core input maps (core c owns heads 2c, 2c+1)."""
    x = np.asarray(x, dtype=np.float32)
    w_qkv = np.asarray(w_qkv, dtype=np.float32)
    w_o = np.asarray(w_o, dtype=np.float32)
    xT = np.ascontiguousarray(x.reshape(R, D).T).astype(ml_dtypes.bfloat16)
    consts = host_constants()
    maps = []
    for c in range(NCORES):
        h0 = NHL * c
        qrows = w_qkv[h0 * HD:(h0 + NHL) * HD]                  # (64, 512)
        krows = w_qkv[D + h0 * HD:D + (h0 + NHL) * HD]
        vrows = w_qkv[2 * D + h0 * HD:2 * D + (h0 + NHL) * HD]
        m = dict(consts)
        m["xT"] = xT
        m["wqkvT"] = np.ascontiguousarray(
            np.concatenate([qrows, krows, vrows], axis=0).T
        ).astype(ml_dtypes.bfloat16)                             # (512, 192)
        m["woT"] = np.ascontiguousarray(
            w_o[:, h0 * HD:(h0 + NHL) * HD].T).astype(ml_dtypes.bfloat16)
        maps.append(m)
    return maps


_PROG = None


def _get_prog():
    global _PROG
    if _PROG is None:
        _PROG = build_program()
    return _PROG


def kernel(x, w_qkv, w_o):
    nc = _get_prog()
    maps = core_inputs(x, w_qkv, w_o)
    res = run_bass_kernel_spmd(nc, maps, list(range(NCORES)))
    acc = np.zeros((R, D), dtype=np.float32)
    for i in range(NCORES):
        acc += res.results[i]["out_part"].astype(np.float32)
    return acc.reshape(B, T, D)
